# revision 25
# baseline (speedup 1.0000x reference)
"""Trainium2 Bass kernel for nn_MemoryTimeUnit.

Math: the reference keeps only Zp[:, :P] and averages over V. By linearity the
whole computation collapses to:
  out[b] = (feat[b]^T @ Wp) + Btot,   feat = [y_fwd^T ; y_bwd^T]  ([2D, P])
  y_fwd  = causal conv of memory[b] with kf (64 taps)          (v-independent)
  y_bwd  = anticausal conv of memory[b] with kb  +  Re[g_b lam_b^{P-t} S_c[b,d]]
  S_c[b,d] = sum_{j,v} lam_b^j/V * ts_embeds[b,j,v,d]   <- only heavy part
All prefix/signal-emb responses fold into the bias table Btot.
Sharding: one batch b per core (8 cores). Tables are host-precomputed from the
per-channel params (no data dependence) and replicated.
"""

import numpy as np

B, P, V, L_P, D = 8, 64, 8, 1024, 256
N = 128          # DFT length for the 64-tap memory convs
NCHUNK = 8       # 1024 j rows / 128

_CACHE = {}
LAST_RESULTS = None


def _make_tables(fwd_nu, fwd_theta, fwd_gr, fwd_gi, bwd_nu, bwd_theta, bwd_gr,
                 bwd_gi, proj_W, proj_b, prefix_emb, signal_emb):
    f64 = np.float64
    lam_f = np.exp(-np.exp(fwd_nu.astype(f64)) + 1j * fwd_theta.astype(f64))
    lam_b = np.exp(-np.exp(bwd_nu.astype(f64)) + 1j * bwd_theta.astype(f64))
    g_f = fwd_gr.astype(f64) + 1j * fwd_gi.astype(f64)
    g_b = bwd_gr.astype(f64) + 1j * bwd_gi.astype(f64)

    tau = np.arange(P)
    kf = np.real(g_f[None, :] * lam_f[None, :] ** tau[:, None])   # [64, D]
    kb = np.real(g_b[None, :] * lam_b[None, :] ** tau[:, None])

    jj = np.arange(L_P)
    lamj = lam_b[None, :] ** jj[:, None]                          # [1024, D]
    W = np.concatenate([np.real(lamj) / V, np.imag(lamj) / V], axis=1)

    tt_ = np.arange(P)
    Afac = g_b[None, :] * lam_b[None, :] ** (P - tt_)[:, None]    # [64, D]
    ArT = np.real(Afac).T                                         # [D, 64]
    AiTn = -np.imag(Afac).T
    AT = np.concatenate([ArT[:128], ArT[128:], AiTn[:128], AiTn[128:]], axis=1)

    f = np.arange(N)
    s = np.arange(N)
    ang = 2 * np.pi * np.outer(f, s) / N
    FrT = np.cos(ang).T
    FiT = (-np.sin(ang)).T
    ang_b = 2 * np.pi * np.outer(f, (P - 1 - s)) / N
    FrbT = np.zeros((N, N)); FibT = np.zeros((N, N))
    FrbT[:P, :] = np.cos(ang_b).T[:P, :]
    FibT[:P, :] = (-np.sin(ang_b)).T[:P, :]
    FCAT = np.concatenate([FrT, FiT, FrbT, FibT], axis=1)         # [128, 512]

    Kf = np.fft.fft(kf, n=N, axis=0)
    Kb = np.fft.fft(kb, n=N, axis=0)
    KCAT = np.concatenate([np.real(Kf), np.imag(Kf),
                           np.real(Kb), np.imag(Kb)], axis=1)     # [128, 1024]

    t64 = np.arange(P)
    angi = 2 * np.pi * np.outer(f, t64) / N
    angib = 2 * np.pi * np.outer(f, (P - 1 - t64)) / N
    FINV = np.concatenate([np.cos(angi) / N, -np.sin(angi) / N,
                           np.cos(angib) / N, -np.sin(angib) / N], axis=1)

    pe = prefix_emb.reshape(-1).astype(f64)
    se = signal_emb.reshape(-1).astype(f64)
    cumkf = np.cumsum(kf, axis=0)
    cumkb = np.cumsum(kb, axis=0)
    y_pe_f = pe[None, :] * cumkf
    y_pe_b = pe[None, :] * cumkb[::-1, :]
    geo = np.sum(lamj, axis=0)
    y_se_b = np.real(Afac * geo[None, :]) * se[None, :]
    Bfeat = np.concatenate([y_pe_f, y_pe_b + y_se_b], axis=1)     # [64, 2D]
    BT = proj_b.astype(f64)[None, :] + Bfeat @ proj_W.astype(f64).T

    Wp = np.ascontiguousarray(proj_W.astype(f64).T)               # [2D, D]
    WP = np.concatenate([Wp[0:128], Wp[128:256], Wp[256:384], Wp[384:512]],
                        axis=1)                                   # [128, 1024]

    # EBIG[(jl,v), 112+jl] = 1/V ; slice [112-16s : 240-16s] = selector for sub s
    E = np.zeros((128, 240))
    for jl in range(16):
        E[jl * V:(jl + 1) * V, 112 + jl] = 1.0 / V
    W2 = np.concatenate([np.real(lamj) / V, np.imag(lamj) / V], axis=1)

    import ml_dtypes
    bh = ml_dtypes.bfloat16
    Wp2 = np.concatenate([W2[128 * g:128 * (g + 1), :] for g in range(8)],
                         axis=1)                                  # [128, 4096]
    c = np.float32
    h = np.float16
    return {"W": Wp2.astype(bh), "E": E.astype(bh), "FCAT": FCAT.astype(h),
            "KCAT": KCAT.astype(h), "FINV": FINV.astype(h), "AT": AT.astype(h),
            "WP": WP.astype(h), "BT": BT.astype(c)}


def _build_bass():
    import concourse.bacc as bacc
    import concourse.mybir as mybir
    from concourse.tile import TileContext

    dt = mybir.dt.float32
    nc = bacc.Bacc("TRN2", num_swdge_queues=2)

    ts = nc.dram_tensor("ts", (L_P, V * D), dt, kind="ExternalInput")
    mem = nc.dram_tensor("mem", (N, D), dt, kind="ExternalInput")
    dth = mybir.dt.float16
    dtb = mybir.dt.bfloat16
    Wd = nc.dram_tensor("W", (128, 16 * D), dtb, kind="ExternalInput")
    Ed = nc.dram_tensor("E", (128, 240), dtb, kind="ExternalInput")
    FCATd = nc.dram_tensor("FCAT", (N, 4 * N), dth, kind="ExternalInput")
    KCATd = nc.dram_tensor("KCAT", (N, 4 * D), dth, kind="ExternalInput")
    FINVd = nc.dram_tensor("FINV", (N, 4 * P), dth, kind="ExternalInput")
    ATd = nc.dram_tensor("AT", (N, 4 * P), dth, kind="ExternalInput")
    WPd = nc.dram_tensor("WP", (N, 4 * D), dth, kind="ExternalInput")
    BTd = nc.dram_tensor("BT", (P, D), dt, kind="ExternalInput")
    outd = nc.dram_tensor("out", (P, D), dt, kind="ExternalOutput")

    with TileContext(nc) as tc:
        with (
            tc.tile_pool(name="xin", bufs=5) as xin_pool,
            tc.tile_pool(name="work", bufs=3) as work_pool,
            tc.tile_pool(name="pp", bufs=8) as p_pool,
            tc.tile_pool(name="const", bufs=1) as const_pool,
            tc.tile_pool(name="ps", bufs=1, space="PSUM") as ps_pool,
            tc.tile_pool(name="psz", bufs=1, space="PSUM") as psz_pool,
        ):
            # --- tables needed by the memory-conv path first
            x0 = xin_pool.tile([128, V * D], dt, tag="x0")
            nc.sync.dma_start(out=x0[:], in_=ts[0:128, :])
            fcat = const_pool.tile([N, 4 * N], dth)
            nc.scalar.dma_start(out=fcat[:], in_=FCATd[:])
            kcat = const_pool.tile([N, 4 * D], dth)
            nc.scalar.dma_start(out=kcat[:], in_=KCATd[:])
            finv = const_pool.tile([N, 4 * P], dth)
            nc.scalar.dma_start(out=finv[:], in_=FINVd[:])
            ones = const_pool.tile([128, 1], dt)
            nc.vector.memset(ones[:], 1.0)
            ones_h = const_pool.tile([128, 1], dtb)
            nc.vector.memset(ones_h[:], 1.0)
            w_all = const_pool.tile([128, 16 * D], dtb)
            nc.scalar.dma_start(out=w_all[:], in_=Wd[:])

            s_psum = ps_pool.tile([1, 2 * D], dt)

            def emit_chunk(g):
                dte = dt if g == 0 else dtb
                if g == 0:
                    x = x0
                else:
                    x = xin_pool.tile([128, V * D], dtb, tag="x")
                    nc.gpsimd.dma_start(out=x[:], in_=ts[128 * g:128 * (g + 1), :])
                a4 = work_pool.tile([128, 4 * D], dte, tag="a4" + ("f" if g == 0 else ""))
                nc.vector.tensor_add(out=a4[:], in0=x[:, 0:4 * D],
                                     in1=x[:, 4 * D:8 * D])
                a2 = work_pool.tile([128, 2 * D], dte, tag="a2" + ("f" if g == 0 else ""))
                nc.vector.tensor_add(out=a2[:], in0=a4[:, 0:2 * D],
                                     in1=a4[:, 2 * D:4 * D])
                a1 = work_pool.tile([128, D], dte, tag="a1" + ("f" if g == 0 else ""))
                nc.vector.tensor_add(out=a1[:], in0=a2[:, 0:D], in1=a2[:, D:2 * D])
                wt = w_all[:, 2 * D * g:2 * D * (g + 1)]
                p = p_pool.tile([128, 2 * D], dtb, tag="p")
                nc.vector.tensor_mul(out=p[:, 0:D], in0=a1[:], in1=wt[:, 0:D])
                nc.vector.tensor_mul(out=p[:, D:2 * D], in0=a1[:],
                                     in1=wt[:, D:2 * D])
                nc.tensor.matmul(s_psum[:], ones_h[:], p[:],
                                 start=(g == 0), stop=(g == NCHUNK - 1))

            emit_chunk(0)
            mp = const_pool.tile([N, D], dth)
            nc.gpsimd.dma_start(out=mp[:], in_=mem[:])
            emit_chunk(1)

            # --- memory DFT path (scheduled among early chunks)
            psum_f = psz_pool.tile([N, 2 * D], dt)
            psum_b = psz_pool.tile([N, 2 * D], dt)
            for h, pt in ((0, psum_f), (1, psum_b)):
                nc.tensor.matmul(pt[:, 0:D], fcat[:, 2 * N * h:2 * N * h + N],
                                 mp[:], start=True, stop=True)
                nc.tensor.matmul(pt[:, D:2 * D],
                                 fcat[:, 2 * N * h + N:2 * N * h + 2 * N],
                                 mp[:], start=True, stop=True)
            y_f = const_pool.tile([N, 2 * D], dth)
            y_b = const_pool.tile([N, 2 * D], dth)
            for pt, yt, ko in ((psum_f, y_f, 0), (psum_b, y_b, 2 * D)):
                tmp = work_pool.tile([N, D], dt, tag="ptmp")
                zr, zi = pt[:, 0:D], pt[:, D:2 * D]
                kr, ki = kcat[:, ko:ko + D], kcat[:, ko + D:ko + 2 * D]
                nc.vector.tensor_mul(out=yt[:, 0:D], in0=zr, in1=kr)
                nc.vector.tensor_mul(out=tmp[:], in0=zi, in1=ki)
                nc.vector.tensor_sub(out=yt[:, 0:D], in0=yt[:, 0:D], in1=tmp[:])
                tmp2 = work_pool.tile([N, D], dt, tag="ptmp")
                nc.vector.tensor_mul(out=yt[:, D:2 * D], in0=zr, in1=ki)
                nc.vector.tensor_mul(out=tmp2[:], in0=zi, in1=kr)
                nc.vector.tensor_add(out=yt[:, D:2 * D], in0=yt[:, D:2 * D],
                                     in1=tmp2[:])
            featT = psz_pool.tile([128, 4 * P], dt)
            for di, (yt, fo) in enumerate(((y_f, 0), (y_b, 2 * P))):
                for h in range(2):
                    o = 2 * P * di + P * h
                    nc.tensor.matmul(featT[:, o:o + P],
                                     yt[:, 128 * h:128 * h + 128],
                                     finv[:, fo:fo + P], start=True, stop=False)
                    nc.tensor.matmul(featT[:, o:o + P],
                                     yt[:, D + 128 * h:D + 128 * h + 128],
                                     finv[:, fo + P:fo + 2 * P],
                                     start=False, stop=True)

            # tables for the tail sections (scalar queue, after the early ones)
            at = const_pool.tile([N, 4 * P], dth)
            nc.scalar.dma_start(out=at[:], in_=ATd[:])
            wp = const_pool.tile([N, 4 * D], dth)
            nc.scalar.dma_start(out=wp[:], in_=WPd[:])
            bt = const_pool.tile([P, D], dt)
            nc.scalar.dma_start(out=bt[:], in_=BTd[:])

            for g in range(2, NCHUNK):
                emit_chunk(g)

            # --- S -> sbuf -> per-d columns
            s_sb = const_pool.tile([1, 2 * D], dt)
            nc.vector.tensor_copy(out=s_sb[:], in_=s_psum[:])
            st_psum = ps_pool.tile([128, 4], dt)
            for g in range(4):
                nc.tensor.matmul(st_psum[:, g:g + 1],
                                 s_sb[0:1, 128 * g:128 * (g + 1)],
                                 ones[0:1, 0:1], start=True, stop=True)


            # feat sbuf: fwd copy; bwd = featT + ArT*Sr + AiTn*Si
            feat = const_pool.tile([128, 4 * P], dth)
            nc.vector.tensor_copy(out=feat[:, 0:2 * P], in_=featT[:, 0:2 * P])
            for h in range(2):
                ua = work_pool.tile([128, P], dt, tag="sig")
                ub = work_pool.tile([128, P], dt, tag="sig")
                nc.vector.tensor_scalar_mul(ua[:], at[:, P * h:P * h + P],
                                            st_psum[:, h:h + 1])
                nc.vector.tensor_scalar_mul(ub[:], at[:, 2 * P + P * h:3 * P + P * h],
                                            st_psum[:, 2 + h:3 + h])
                nc.vector.tensor_add(out=ua[:], in0=ua[:], in1=ub[:])
                o = 2 * P + P * h
                nc.vector.tensor_add(out=feat[:, o:o + P], in0=featT[:, o:o + P],
                                     in1=ua[:])

            # proj + bias + out
            proj_psum = ps_pool.tile([P, D], dt)
            for g in range(4):
                nc.tensor.matmul(proj_psum[:], feat[:, P * g:P * (g + 1)],
                                 wp[:, D * g:D * (g + 1)],
                                 start=(g == 0), stop=(g == 3))
            out_sb = const_pool.tile([P, D], dt)
            nc.vector.tensor_add(out=out_sb[:], in0=proj_psum[:], in1=bt[:])
            nc.scalar.dma_start(out=outd[:], in_=out_sb[:])

    nc.compile()
    return nc


def kernel(**inputs):
    global LAST_RESULTS
    import os
    from concourse.bass_utils import run_bass_kernel_spmd

    if "nc" not in _CACHE:
        _CACHE["nc"] = _build_bass()
    nc = _CACHE["nc"]

    pkeys = ["fwd_nu", "fwd_theta", "fwd_gr", "fwd_gi", "bwd_nu", "bwd_theta",
             "bwd_gr", "bwd_gi", "proj_W", "proj_b", "prefix_emb", "signal_emb"]
    tables = _make_tables(**{k: np.asarray(inputs[k]) for k in pkeys})

    memory = np.ascontiguousarray(np.asarray(inputs["memory"], np.float32))
    ts_embeds = np.ascontiguousarray(np.asarray(inputs["ts_embeds"], np.float32))

    in_maps = []
    for b in range(B):
        memp = np.zeros((N, D), np.float32)
        memp[:P] = memory[b]
        m = {"ts": ts_embeds[b].reshape(L_P, V * D), "mem": memp}
        m.update(tables)
        in_maps.append(m)

    trace = os.environ.get("BASS_KERNEL_TRACE", "0") == "1"
    res = run_bass_kernel_spmd(nc, in_maps, core_ids=list(range(B)), trace=trace)
    LAST_RESULTS = res
    return np.stack([res.results[b]["out"] for b in range(B)], axis=0)


# revision 26
# speedup vs baseline: 1.0118x; 1.0118x over previous
"""Trainium2 Bass kernel for nn_MemoryTimeUnit.

Math: the reference keeps only Zp[:, :P] and averages over V. By linearity the
whole computation collapses to:
  out[b] = (feat[b]^T @ Wp) + Btot,   feat = [y_fwd^T ; y_bwd^T]  ([2D, P])
  y_fwd  = causal conv of memory[b] with kf (64 taps)          (v-independent)
  y_bwd  = anticausal conv of memory[b] with kb  +  Re[g_b lam_b^{P-t} S_c[b,d]]
  S_c[b,d] = sum_{j,v} lam_b^j/V * ts_embeds[b,j,v,d]   <- only heavy part
All prefix/signal-emb responses fold into the bias table Btot.
Sharding: one batch b per core (8 cores). Tables are host-precomputed from the
per-channel params (no data dependence) and replicated.
"""

import numpy as np

B, P, V, L_P, D = 8, 64, 8, 1024, 256
N = 128          # DFT length for the 64-tap memory convs
NCHUNK = 8       # 1024 j rows / 128

_CACHE = {}
LAST_RESULTS = None


def _make_tables(fwd_nu, fwd_theta, fwd_gr, fwd_gi, bwd_nu, bwd_theta, bwd_gr,
                 bwd_gi, proj_W, proj_b, prefix_emb, signal_emb):
    f64 = np.float64
    lam_f = np.exp(-np.exp(fwd_nu.astype(f64)) + 1j * fwd_theta.astype(f64))
    lam_b = np.exp(-np.exp(bwd_nu.astype(f64)) + 1j * bwd_theta.astype(f64))
    g_f = fwd_gr.astype(f64) + 1j * fwd_gi.astype(f64)
    g_b = bwd_gr.astype(f64) + 1j * bwd_gi.astype(f64)

    tau = np.arange(P)
    kf = np.real(g_f[None, :] * lam_f[None, :] ** tau[:, None])   # [64, D]
    kb = np.real(g_b[None, :] * lam_b[None, :] ** tau[:, None])

    jj = np.arange(L_P)
    lamj = lam_b[None, :] ** jj[:, None]                          # [1024, D]
    W = np.concatenate([np.real(lamj) / V, np.imag(lamj) / V], axis=1)

    tt_ = np.arange(P)
    Afac = g_b[None, :] * lam_b[None, :] ** (P - tt_)[:, None]    # [64, D]
    ArT = np.real(Afac).T                                         # [D, 64]
    AiTn = -np.imag(Afac).T
    AT = np.concatenate([ArT[:128], ArT[128:], AiTn[:128], AiTn[128:]], axis=1)

    f = np.arange(N)
    s = np.arange(N)
    ang = 2 * np.pi * np.outer(f, s) / N
    FrT = np.cos(ang).T
    FiT = (-np.sin(ang)).T
    ang_b = 2 * np.pi * np.outer(f, (P - 1 - s)) / N
    FrbT = np.zeros((N, N)); FibT = np.zeros((N, N))
    FrbT[:P, :] = np.cos(ang_b).T[:P, :]
    FibT[:P, :] = (-np.sin(ang_b)).T[:P, :]
    FCAT = np.concatenate([FrT, FiT, FrbT, FibT], axis=1)         # [128, 512]

    Kf = np.fft.fft(kf, n=N, axis=0)
    Kb = np.fft.fft(kb, n=N, axis=0)
    KCAT = np.concatenate([np.real(Kf), np.imag(Kf),
                           np.real(Kb), np.imag(Kb)], axis=1)     # [128, 1024]

    t64 = np.arange(P)
    angi = 2 * np.pi * np.outer(f, t64) / N
    angib = 2 * np.pi * np.outer(f, (P - 1 - t64)) / N
    FINV = np.concatenate([np.cos(angi) / N, -np.sin(angi) / N,
                           np.cos(angib) / N, -np.sin(angib) / N], axis=1)

    pe = prefix_emb.reshape(-1).astype(f64)
    se = signal_emb.reshape(-1).astype(f64)
    cumkf = np.cumsum(kf, axis=0)
    cumkb = np.cumsum(kb, axis=0)
    y_pe_f = pe[None, :] * cumkf
    y_pe_b = pe[None, :] * cumkb[::-1, :]
    geo = np.sum(lamj, axis=0)
    y_se_b = np.real(Afac * geo[None, :]) * se[None, :]
    Bfeat = np.concatenate([y_pe_f, y_pe_b + y_se_b], axis=1)     # [64, 2D]
    BT = proj_b.astype(f64)[None, :] + Bfeat @ proj_W.astype(f64).T

    Wp = np.ascontiguousarray(proj_W.astype(f64).T)               # [2D, D]
    WP = np.concatenate([Wp[0:128], Wp[128:256], Wp[256:384], Wp[384:512]],
                        axis=1)                                   # [128, 1024]

    # EBIG[(jl,v), 112+jl] = 1/V ; slice [112-16s : 240-16s] = selector for sub s
    E = np.zeros((128, 240))
    for jl in range(16):
        E[jl * V:(jl + 1) * V, 112 + jl] = 1.0 / V
    W2 = np.concatenate([np.real(lamj) / V, np.imag(lamj) / V], axis=1)

    import ml_dtypes
    bh = ml_dtypes.bfloat16
    Wp2 = np.concatenate([W2[128 * g:128 * (g + 1), :] for g in range(8)],
                         axis=1)                                  # [128, 4096]
    c = np.float32
    h = np.float16
    return {"W": Wp2.astype(bh), "E": E.astype(bh), "FCAT": FCAT.astype(h),
            "KCAT": KCAT.astype(h), "FINV": FINV.astype(h), "AT": AT.astype(h),
            "WP": WP.astype(h), "BT": BT.astype(c)}


def _build_bass():
    import concourse.bacc as bacc
    import concourse.mybir as mybir
    from concourse.tile import TileContext

    dt = mybir.dt.float32
    nc = bacc.Bacc("TRN2", num_swdge_queues=3)

    ts = nc.dram_tensor("ts", (L_P, V * D), dt, kind="ExternalInput")
    mem = nc.dram_tensor("mem", (N, D), dt, kind="ExternalInput")
    dth = mybir.dt.float16
    dtb = mybir.dt.bfloat16
    Wd = nc.dram_tensor("W", (128, 16 * D), dtb, kind="ExternalInput")
    Ed = nc.dram_tensor("E", (128, 240), dtb, kind="ExternalInput")
    FCATd = nc.dram_tensor("FCAT", (N, 4 * N), dth, kind="ExternalInput")
    KCATd = nc.dram_tensor("KCAT", (N, 4 * D), dth, kind="ExternalInput")
    FINVd = nc.dram_tensor("FINV", (N, 4 * P), dth, kind="ExternalInput")
    ATd = nc.dram_tensor("AT", (N, 4 * P), dth, kind="ExternalInput")
    WPd = nc.dram_tensor("WP", (N, 4 * D), dth, kind="ExternalInput")
    BTd = nc.dram_tensor("BT", (P, D), dt, kind="ExternalInput")
    outd = nc.dram_tensor("out", (P, D), dt, kind="ExternalOutput")

    with TileContext(nc) as tc:
        with (
            tc.tile_pool(name="xin", bufs=5) as xin_pool,
            tc.tile_pool(name="work", bufs=3) as work_pool,
            tc.tile_pool(name="pp", bufs=8) as p_pool,
            tc.tile_pool(name="const", bufs=1) as const_pool,
            tc.tile_pool(name="ps", bufs=1, space="PSUM") as ps_pool,
            tc.tile_pool(name="psz", bufs=1, space="PSUM") as psz_pool,
        ):
            # --- tables needed by the memory-conv path first
            x0 = xin_pool.tile([128, V * D], dtb, tag="x")
            nc.gpsimd.dma_start(out=x0[:], in_=ts[0:128, :])
            fcat = const_pool.tile([N, 4 * N], dth)
            nc.scalar.dma_start(out=fcat[:], in_=FCATd[:])
            kcat = const_pool.tile([N, 4 * D], dth)
            nc.scalar.dma_start(out=kcat[:], in_=KCATd[:])
            finv = const_pool.tile([N, 4 * P], dth)
            nc.scalar.dma_start(out=finv[:], in_=FINVd[:])
            ones = const_pool.tile([128, 1], dt)
            nc.vector.memset(ones[:], 1.0)
            ones_h = const_pool.tile([128, 1], dtb)
            nc.vector.memset(ones_h[:], 1.0)
            w_all = const_pool.tile([128, 16 * D], dtb)
            nc.scalar.dma_start(out=w_all[:], in_=Wd[:])

            s_psum = ps_pool.tile([1, 2 * D], dt)

            def emit_chunk(g):
                dte = dtb
                if g == 0:
                    x = x0
                else:
                    x = xin_pool.tile([128, V * D], dtb, tag="x")
                    nc.gpsimd.dma_start(out=x[:], in_=ts[128 * g:128 * (g + 1), :])
                a4 = work_pool.tile([128, 4 * D], dte, tag="a4")
                nc.vector.tensor_add(out=a4[:], in0=x[:, 0:4 * D],
                                     in1=x[:, 4 * D:8 * D])
                a2 = work_pool.tile([128, 2 * D], dte, tag="a2")
                nc.vector.tensor_add(out=a2[:], in0=a4[:, 0:2 * D],
                                     in1=a4[:, 2 * D:4 * D])
                a1 = work_pool.tile([128, D], dte, tag="a1")
                nc.vector.tensor_add(out=a1[:], in0=a2[:, 0:D], in1=a2[:, D:2 * D])
                wt = w_all[:, 2 * D * g:2 * D * (g + 1)]
                p = p_pool.tile([128, 2 * D], dtb, tag="p")
                nc.vector.tensor_mul(out=p[:, 0:D], in0=a1[:], in1=wt[:, 0:D])
                nc.vector.tensor_mul(out=p[:, D:2 * D], in0=a1[:],
                                     in1=wt[:, D:2 * D])
                nc.tensor.matmul(s_psum[:], ones_h[:], p[:],
                                 start=(g == 0), stop=(g == NCHUNK - 1))

            emit_chunk(0)
            mp = const_pool.tile([N, D], dth)
            nc.gpsimd.dma_start(out=mp[:], in_=mem[:])
            emit_chunk(1)

            # --- memory DFT path (scheduled among early chunks)
            psum_f = psz_pool.tile([N, 2 * D], dt)
            psum_b = psz_pool.tile([N, 2 * D], dt)
            for h, pt in ((0, psum_f), (1, psum_b)):
                nc.tensor.matmul(pt[:, 0:D], fcat[:, 2 * N * h:2 * N * h + N],
                                 mp[:], start=True, stop=True)
                nc.tensor.matmul(pt[:, D:2 * D],
                                 fcat[:, 2 * N * h + N:2 * N * h + 2 * N],
                                 mp[:], start=True, stop=True)
            y_f = const_pool.tile([N, 2 * D], dth)
            y_b = const_pool.tile([N, 2 * D], dth)
            for pt, yt, ko in ((psum_f, y_f, 0), (psum_b, y_b, 2 * D)):
                tmp = work_pool.tile([N, D], dt, tag="ptmp")
                zr, zi = pt[:, 0:D], pt[:, D:2 * D]
                kr, ki = kcat[:, ko:ko + D], kcat[:, ko + D:ko + 2 * D]
                nc.vector.tensor_mul(out=yt[:, 0:D], in0=zr, in1=kr)
                nc.vector.tensor_mul(out=tmp[:], in0=zi, in1=ki)
                nc.vector.tensor_sub(out=yt[:, 0:D], in0=yt[:, 0:D], in1=tmp[:])
                tmp2 = work_pool.tile([N, D], dt, tag="ptmp")
                nc.vector.tensor_mul(out=yt[:, D:2 * D], in0=zr, in1=ki)
                nc.vector.tensor_mul(out=tmp2[:], in0=zi, in1=kr)
                nc.vector.tensor_add(out=yt[:, D:2 * D], in0=yt[:, D:2 * D],
                                     in1=tmp2[:])
            featT = psz_pool.tile([128, 4 * P], dt)
            for di, (yt, fo) in enumerate(((y_f, 0), (y_b, 2 * P))):
                for h in range(2):
                    o = 2 * P * di + P * h
                    nc.tensor.matmul(featT[:, o:o + P],
                                     yt[:, 128 * h:128 * h + 128],
                                     finv[:, fo:fo + P], start=True, stop=False)
                    nc.tensor.matmul(featT[:, o:o + P],
                                     yt[:, D + 128 * h:D + 128 * h + 128],
                                     finv[:, fo + P:fo + 2 * P],
                                     start=False, stop=True)

            # tables for the tail sections (scalar queue, after the early ones)
            at = const_pool.tile([N, 4 * P], dth)
            nc.scalar.dma_start(out=at[:], in_=ATd[:])
            wp = const_pool.tile([N, 4 * D], dth)
            nc.scalar.dma_start(out=wp[:], in_=WPd[:])
            bt = const_pool.tile([P, D], dt)
            nc.scalar.dma_start(out=bt[:], in_=BTd[:])

            for g in range(2, NCHUNK):
                emit_chunk(g)

            # --- S -> sbuf -> per-d columns
            s_sb = const_pool.tile([1, 2 * D], dt)
            nc.vector.tensor_copy(out=s_sb[:], in_=s_psum[:])
            st_psum = ps_pool.tile([128, 4], dt)
            for g in range(4):
                nc.tensor.matmul(st_psum[:, g:g + 1],
                                 s_sb[0:1, 128 * g:128 * (g + 1)],
                                 ones[0:1, 0:1], start=True, stop=True)


            # feat sbuf: fwd copy; bwd = featT + ArT*Sr + AiTn*Si
            feat = const_pool.tile([128, 4 * P], dth)
            nc.vector.tensor_copy(out=feat[:, 0:2 * P], in_=featT[:, 0:2 * P])
            for h in range(2):
                ua = work_pool.tile([128, P], dt, tag="sig")
                ub = work_pool.tile([128, P], dt, tag="sig")
                nc.vector.tensor_scalar_mul(ua[:], at[:, P * h:P * h + P],
                                            st_psum[:, h:h + 1])
                nc.vector.tensor_scalar_mul(ub[:], at[:, 2 * P + P * h:3 * P + P * h],
                                            st_psum[:, 2 + h:3 + h])
                nc.vector.tensor_add(out=ua[:], in0=ua[:], in1=ub[:])
                o = 2 * P + P * h
                nc.vector.tensor_add(out=feat[:, o:o + P], in0=featT[:, o:o + P],
                                     in1=ua[:])

            # proj + bias + out
            proj_psum = ps_pool.tile([P, D], dt)
            for g in range(4):
                nc.tensor.matmul(proj_psum[:], feat[:, P * g:P * (g + 1)],
                                 wp[:, D * g:D * (g + 1)],
                                 start=(g == 0), stop=(g == 3))
            out_sb = const_pool.tile([P, D], dt)
            nc.vector.tensor_add(out=out_sb[:], in0=proj_psum[:], in1=bt[:])
            nc.scalar.dma_start(out=outd[:], in_=out_sb[:])

    nc.compile()
    return nc


def kernel(**inputs):
    global LAST_RESULTS
    import os
    from concourse.bass_utils import run_bass_kernel_spmd

    if "nc" not in _CACHE:
        _CACHE["nc"] = _build_bass()
    nc = _CACHE["nc"]

    pkeys = ["fwd_nu", "fwd_theta", "fwd_gr", "fwd_gi", "bwd_nu", "bwd_theta",
             "bwd_gr", "bwd_gi", "proj_W", "proj_b", "prefix_emb", "signal_emb"]
    tables = _make_tables(**{k: np.asarray(inputs[k]) for k in pkeys})

    memory = np.ascontiguousarray(np.asarray(inputs["memory"], np.float32))
    ts_embeds = np.ascontiguousarray(np.asarray(inputs["ts_embeds"], np.float32))

    in_maps = []
    for b in range(B):
        memp = np.zeros((N, D), np.float32)
        memp[:P] = memory[b]
        m = {"ts": ts_embeds[b].reshape(L_P, V * D), "mem": memp}
        m.update(tables)
        in_maps.append(m)

    trace = os.environ.get("BASS_KERNEL_TRACE", "0") == "1"
    res = run_bass_kernel_spmd(nc, in_maps, core_ids=list(range(B)), trace=trace)
    LAST_RESULTS = res
    return np.stack([res.results[b]["out"] for b in range(B)], axis=0)
